# revision 6
# baseline (speedup 1.0000x reference)
"""Trainium2 Bass kernel for nn_BertAdapterAttentionMask.

Math restructuring (validated vs reference in fp64):
  * The query comes from a broadcast task embedding -> q is one [H] vector,
    constant over (b, s). The K projection therefore collapses to a rank-hd
    GEMM:  scores[., d] = hid @ Mk[d, :] + ck[d], Mk = fold(q, gk, k_w)/sqrt(hd).
  * Per-task adapter gates fold into fc2 weights (host side).
  * softmax(scores) sums to 1 over tasks, so the V bias contributes a constant
    vector; it is folded into the residual x on the host.
  * scores GEMM uses a column-duplicated Mk (M=128) so probs come out
    partition-duplicated for free (broadcast over the 2x64 row halves).

Per-core layout: fully "transposed" pipeline (features on partitions,
sequence on the free axis). Data-parallel over batch B=8 across 8 cores.
All GEMM operands bf16 (1 cyc/row on PE), fp32 accumulation + softmax.
"""

import numpy as np
import ml_dtypes
from contextlib import ExitStack

import concourse.bass as bass
import concourse.tile as tile
from concourse import bacc, mybir
from concourse.bass_utils import run_bass_kernel_spmd
from concourse.masks import make_identity

AF = mybir.ActivationFunctionType
BF16 = mybir.dt.bfloat16
F32 = mybir.dt.float32
NPBF16 = ml_dtypes.bfloat16

B, S, H, A, NH, HD = 8, 2048, 1024, 512, 16, 64
T = 6              # tasks = t + 1
P = 128
ST = 512           # s-tile (free-dim tile)
NST = S // ST      # 4
NHC = H // P       # 8 h-chunks
NAC = A // P       # 4 a-chunks
SMAX = 400.0

_CACHE = {}


def _build_nc():
    nc = bacc.Bacc("TRN2", target_bir_lowering=False, debug=False)

    d_xT = nc.dram_tensor("xT", [H, S], BF16, kind="ExternalInput").ap()
    d_xres = nc.dram_tensor("xres", [S, H], F32, kind="ExternalInput").ap()
    d_fc1T = nc.dram_tensor("fc1T", [H, A], BF16, kind="ExternalInput").ap()
    d_fc1b = nc.dram_tensor("fc1b", [NAC, P, 1], F32, kind="ExternalInput").ap()
    d_W2T = nc.dram_tensor("W2T", [T, A, H], BF16, kind="ExternalInput").ap()
    d_fc2b = nc.dram_tensor("fc2b", [NHC, P, 1], F32, kind="ExternalInput").ap()
    d_Mk = nc.dram_tensor("MkT", [H, P], BF16, kind="ExternalInput").ap()
    d_ck = nc.dram_tensor("ck", [P, 1], F32, kind="ExternalInput").ap()
    d_Wv = nc.dram_tensor("WvT", [H, H], BF16, kind="ExternalInput").ap()
    d_g2 = nc.dram_tensor("g2sb", [P, NHC * T], F32, kind="ExternalInput").ap()
    d_out = nc.dram_tensor("out", [S, H], F32, kind="ExternalOutput").ap()

    with tile.TileContext(nc) as tc:
        with ExitStack() as ctx:
            wp = ctx.enter_context(tc.tile_pool(name="weights", bufs=1))
            xp = ctx.enter_context(tc.tile_pool(name="acts", bufs=2))
            psp = ctx.enter_context(
                tc.tile_pool(name="psum", bufs=2, space="PSUM")
            )

            # ---- resident weights ----
            w1 = []
            for k in range(NHC):
                t_ = wp.tile([P, A], BF16, tag=f"w1_{k}")
                nc.sync.dma_start(t_[:], d_fc1T[k * P:(k + 1) * P, :])
                w1.append(t_)
            w2 = [[None] * NAC for _ in range(T)]
            for p in range(T):
                for ac in range(NAC):
                    t_ = wp.tile([P, H], BF16, tag=f"w2_{p}_{ac}")
                    nc.sync.dma_start(t_[:], d_W2T[p, ac * P:(ac + 1) * P, :])
                    w2[p][ac] = t_
            wmk = []
            for j in range(NHC):
                t_ = wp.tile([P, P], BF16, tag=f"wmk_{j}")
                nc.sync.dma_start(t_[:], d_Mk[j * P:(j + 1) * P, :])
                wmk.append(t_)
            wv = []
            for j in range(NHC):
                t_ = wp.tile([P, H], BF16, tag=f"wv_{j}")
                nc.sync.dma_start(t_[:], d_Wv[j * P:(j + 1) * P, :])
                wv.append(t_)
            b1 = wp.tile([P, NAC], F32, tag="b1")
            for ac in range(NAC):
                nc.sync.dma_start(b1[:, ac:ac + 1], d_fc1b[ac])
            b2 = wp.tile([P, NHC], F32, tag="b2")
            for hc in range(NHC):
                nc.sync.dma_start(b2[:, hc:hc + 1], d_fc2b[hc])
            ckt = wp.tile([P, 1], F32, tag="ck")
            nc.sync.dma_start(ckt[:], d_ck[:])
            g2t = wp.tile([P, NHC * T], F32, tag="g2")
            nc.sync.dma_start(g2t[:], d_g2[:])
            ident = wp.tile([P, P], BF16, tag="ident")
            make_identity(nc, ident[:])

            for st in range(NST):
                s0 = st * ST
                # ---- load xT chunks ----
                xt = []
                for k in range(NHC):
                    t_ = xp.tile([P, ST], BF16, name=f"xt{k}", tag=f"xt_{k}", bufs=1)
                    nc.sync.dma_start(t_[:], d_xT[k * P:(k + 1) * P, s0:s0 + ST])
                    xt.append(t_)
                # ---- fc1 -> h1T (gelu) ----
                h1 = []
                for ac in range(NAC):
                    ps = psp.tile([P, ST], F32, tag="ps_mm", bufs=3)
                    for k in range(NHC):
                        nc.tensor.matmul(
                            ps[:], w1[k][:, ac * P:(ac + 1) * P], xt[k][:],
                            start=(k == 0), stop=(k == NHC - 1),
                        )
                    t_ = xp.tile([P, ST], BF16, name=f"h1_{ac}", tag=f"h1_{ac}", bufs=2)
                    nc.scalar.activation(t_[:], ps[:], AF.Gelu, bias=b1[:, ac:ac + 1])
                    h1.append(t_)
                # ---- fc2 per task -> gated gelu store ----
                gst = [
                    xp.tile([P, T, ST], BF16, name=f"g{j}", tag=f"g_{j}", bufs=1)
                    for j in range(NHC)
                ]
                for p in range(T):
                    for j in range(NHC):
                        ps = psp.tile([P, ST], F32, tag="ps_mm", bufs=3)
                        for ac in range(NAC):
                            nc.tensor.matmul(
                                ps[:], w2[p][ac][:, j * P:(j + 1) * P], h1[ac][:],
                                start=(ac == 0), stop=(ac == NAC - 1),
                            )
                        nc.scalar.activation(
                            gst[j][:, p, :], ps[:], AF.Gelu, bias=b2[:, j:j + 1]
                        )
                        nc.gpsimd.tensor_scalar_mul(
                            gst[j][:, p, :], gst[j][:, p, :],
                            g2t[:, j * T + p:j * T + p + 1],
                        )
                # ---- scores (batched: one ACT table switch) -> e = exp ----
                e_t = xp.tile([P, T, ST], F32, tag="e", bufs=1)
                for p in range(T):
                    ps_s = psp.tile([P, ST], F32, tag="ps_s", bufs=1)
                    for j in range(NHC):
                        nc.tensor.matmul(
                            ps_s[:], wmk[j][:], gst[j][:, p, :],
                            start=(j == 0), stop=(j == NHC - 1),
                        )
                    nc.scalar.activation(e_t[:, p, :], ps_s[:], AF.Exp, bias=ckt[:])
                # ---- softmax over tasks (no max-sub: |scores| < 1) ----
                d0 = xp.tile([P, ST], F32, tag="den", bufs=3)
                d1 = xp.tile([P, ST], F32, tag="den", bufs=3)
                d2 = xp.tile([P, ST], F32, tag="den", bufs=3)
                nc.vector.tensor_add(d0[:], e_t[:, 0, :], e_t[:, 1, :])
                nc.vector.tensor_add(d1[:], e_t[:, 2, :], e_t[:, 3, :])
                nc.vector.tensor_add(d2[:], e_t[:, 4, :], e_t[:, 5, :])
                nc.vector.tensor_add(d0[:], d0[:], d1[:])
                nc.vector.tensor_add(d0[:], d0[:], d2[:])
                nc.vector.reciprocal(d0[:], d0[:])
                for p in range(T):
                    nc.vector.tensor_mul(e_t[:, p, :], e_t[:, p, :], d0[:])
                # ---- V GEMM + probs-weighted task sum ----
                ctxs = []
                for hc in range(NHC):
                    eng = nc.vector if hc % 2 == 0 else nc.gpsimd
                    sc = []
                    for p in range(T):
                        ps_v = psp.tile([P, ST], F32, tag="ps_v")
                        for j in range(NHC):
                            nc.tensor.matmul(
                                ps_v[:], wv[j][:, hc * P:(hc + 1) * P],
                                gst[j][:, p, :],
                                start=(j == 0), stop=(j == NHC - 1),
                            )
                        t_ = xp.tile([P, ST], BF16, name=f"sc{p}", tag="sc", bufs=8)
                        nc.vector.tensor_mul(t_[:], ps_v[:], e_t[:, p, :])
                        sc.append(t_)
                    eng.tensor_add(sc[0][:], sc[0][:], sc[1][:])
                    eng.tensor_add(sc[2][:], sc[2][:], sc[3][:])
                    eng.tensor_add(sc[4][:], sc[4][:], sc[5][:])
                    eng.tensor_add(sc[0][:], sc[0][:], sc[2][:])
                    cx = xp.tile([P, ST], BF16, tag="ctx", bufs=10)
                    eng.tensor_add(cx[:], sc[0][:], sc[4][:])
                    ctxs.append(cx)
                # ---- transpose + head-permute + residual + store ----
                for sb in range(ST // P):
                    r0 = s0 + sb * P
                    xr = xp.tile([P, H], F32, tag="xr", bufs=2)
                    nc.sync.dma_start(xr[:], d_xres[r0:r0 + P, :])
                    ot = xp.tile([P, H], F32, tag="ot", bufs=2)
                    for h2 in range(2):
                        ps_t = psp.tile([P, ST], BF16, tag="ps_t")
                        for q in range(4):
                            hc = h2 * 4 + q
                            nc.tensor.transpose(
                                ps_t[:, q * P:(q + 1) * P],
                                ctxs[hc][:, sb * P:(sb + 1) * P],
                                ident[:],
                            )
                        # out cols h' = d*16 + h2*8 + c*2 + nl  for psum col (c,nl,d)
                        o_ap = ot[:].rearrange(
                            "p (d h2 c nl) -> p h2 c nl d", d=HD, h2=2, c=4, nl=2
                        )[:, h2]
                        x_ap = xr[:].rearrange(
                            "p (d h2 c nl) -> p h2 c nl d", d=HD, h2=2, c=4, nl=2
                        )[:, h2]
                        p_ap = ps_t[:].rearrange("p (c nl d) -> p c nl d", c=4, nl=2, d=HD)
                        nc.vector.tensor_add(o_ap, p_ap, x_ap)
                    nc.sync.dma_start(d_out[r0:r0 + P, :], ot[:])
    nc.compile()
    return nc


def _sigmoid(x):
    with np.errstate(over="ignore"):
        return 1.0 / (1.0 + np.exp(-x))


def _host_prep(x, fc1_w, fc1_b, fc2_w, fc2_b, efc1, efc2, etask,
               q_w, q_b, k_w, k_b, v_w, v_b, equery, ekey, evalue, t, s):
    f64 = np.float64
    t = int(t)
    s = float(s)
    assert t + 1 == T and x.shape == (B, S, H)
    fc1_w = np.asarray(fc1_w, f64); fc1_b = np.asarray(fc1_b, f64)
    fc2_w = np.asarray(fc2_w, f64); fc2_b = np.asarray(fc2_b, f64)
    efc1 = np.asarray(efc1, f64); efc2 = np.asarray(efc2, f64)
    etask = np.asarray(etask, f64)
    q_w = np.asarray(q_w, f64); q_b = np.asarray(q_b, f64)
    k_w = np.asarray(k_w, f64); k_b = np.asarray(k_b, f64)
    v_w = np.asarray(v_w, f64); v_b = np.asarray(v_b, f64)
    equery = np.asarray(equery, f64); ekey = np.asarray(ekey, f64)
    evalue = np.asarray(evalue, f64)

    g1 = np.stack([_sigmoid(s * efc1[t])] + [_sigmoid(SMAX * efc1[p]) for p in range(t)])
    g2 = np.stack([_sigmoid(s * efc2[t])] + [_sigmoid(SMAX * efc2[p]) for p in range(t)])
    gq = _sigmoid(s * equery[t]); gk = _sigmoid(s * ekey[t]); gv = _sigmoid(s * evalue[t])

    q_vec = (etask[t] @ q_w.T + q_b) * gq
    q_mat = q_vec.reshape(NH, HD)
    kwg = k_w * gk[:, None]
    Mk = np.einsum("nd,ndj->dj", q_mat, kwg.reshape(NH, HD, H)) / np.sqrt(HD)
    ck = np.einsum("nd,nd->d", q_mat, (k_b * gk).reshape(NH, HD)) / np.sqrt(HD)

    MkTdup = np.ascontiguousarray(
        np.concatenate([Mk.T, Mk.T], axis=1).astype(NPBF16))       # [H,128]
    ck_dup = np.tile(ck, 2).astype(np.float32).reshape(P, 1)
    W2T = np.ascontiguousarray(
        (fc2_w.T[None] * g1[:, :, None]).astype(NPBF16))           # [T,A,H]
    WvT = np.ascontiguousarray((v_w * gv[:, None]).T.astype(NPBF16))  # [H,H]
    vbg_perm = (v_b * gv).reshape(NH, HD).T.reshape(H)             # h' = d*16+n
    fc1T = np.ascontiguousarray(fc1_w.T.astype(NPBF16))            # [H,A]
    fc1b = fc1_b.astype(np.float32).reshape(NAC, P, 1)
    fc2b = fc2_b.astype(np.float32).reshape(NHC, P, 1)
    # g2sb[r, j*T+p] = g2[p, j*128+r]
    g2sb = np.ascontiguousarray(
        g2.reshape(T, NHC, P).transpose(2, 1, 0).reshape(P, NHC * T).astype(np.float32))

    shared = dict(fc1T=fc1T, fc1b=fc1b, W2T=W2T, fc2b=fc2b,
                  MkT=MkTdup, ck=ck_dup, WvT=WvT, g2sb=g2sb)
    per_core = []
    x32 = np.asarray(x, np.float32)
    xres_all = x32 + vbg_perm.astype(np.float32)[None, None, :]
    for b_ in range(B):
        m = dict(shared)
        m["xT"] = np.ascontiguousarray(x32[b_].T.astype(NPBF16))
        m["xres"] = np.ascontiguousarray(xres_all[b_])
        per_core.append(m)
    return per_core


def kernel(**inputs):
    if "nc" not in _CACHE:
        _CACHE["nc"] = _build_nc()
    nc = _CACHE["nc"]
    in_maps = _host_prep(**inputs)
    res = run_bass_kernel_spmd(nc, in_maps, core_ids=list(range(B)))
    out = np.stack([res.results[c]["out"] for c in range(B)], axis=0)
    return out.astype(np.float32)


# revision 7
# speedup vs baseline: 2.7384x; 2.7384x over previous
"""Trainium2 Bass kernel for nn_BertAdapterAttentionMask.

Math restructuring (validated vs reference in fp64):
  * The query comes from a broadcast task embedding -> q is one [H] vector,
    constant over (b, s). The K projection therefore collapses to a rank-hd
    GEMM:  scores[., d] = hid @ Mk[d, :] + ck[d], Mk = fold(q, gk, k_w)/sqrt(hd).
  * Per-task adapter gates fold into fc2 weights (host side).
  * softmax(scores) sums to 1 over tasks, so the V bias contributes a constant
    vector; it is folded into the residual x on the host.
  * scores GEMM uses a column-duplicated Mk (M=128) so probs come out
    partition-duplicated for free (broadcast over the 2x64 row halves).

Per-core layout: fully "transposed" pipeline (features on partitions,
sequence on the free axis). Data-parallel over batch B=8 across 8 cores.
All GEMM operands bf16 (1 cyc/row on PE), fp32 accumulation + softmax.
"""

import numpy as np
import ml_dtypes
from contextlib import ExitStack

import concourse.bass as bass
import concourse.tile as tile
from concourse import bacc, mybir
from concourse.bass_utils import run_bass_kernel_spmd
from concourse.masks import make_identity

AF = mybir.ActivationFunctionType
BF16 = mybir.dt.bfloat16
F32 = mybir.dt.float32
NPBF16 = ml_dtypes.bfloat16

B, S, H, A, NH, HD = 8, 2048, 1024, 512, 16, 64
T = 6              # tasks = t + 1
P = 128
ST = 512           # s-tile (free-dim tile)
NST = S // ST      # 4
NHC = H // P       # 8 h-chunks
NAC = A // P       # 4 a-chunks
SMAX = 400.0

_CACHE = {}


def _build_nc():
    nc = bacc.Bacc("TRN2", target_bir_lowering=False, debug=False)

    d_xT = nc.dram_tensor("xT", [H, S], BF16, kind="ExternalInput").ap()
    d_xres = nc.dram_tensor("xres", [S, H], F32, kind="ExternalInput").ap()
    d_fc1T = nc.dram_tensor("fc1T", [H, A], BF16, kind="ExternalInput").ap()
    d_fc1b = nc.dram_tensor("fc1b", [NAC, P, 1], F32, kind="ExternalInput").ap()
    d_W2T = nc.dram_tensor("W2T", [T, A, H], BF16, kind="ExternalInput").ap()
    d_fc2b = nc.dram_tensor("fc2b", [NHC, P, 1], F32, kind="ExternalInput").ap()
    d_Mk = nc.dram_tensor("MkT", [H, P], BF16, kind="ExternalInput").ap()
    d_ck = nc.dram_tensor("ck", [P, 1], F32, kind="ExternalInput").ap()
    d_Wv = nc.dram_tensor("WvT", [H, H], BF16, kind="ExternalInput").ap()
    d_g2 = nc.dram_tensor("g2sb", [P, NHC * T], F32, kind="ExternalInput").ap()
    d_out = nc.dram_tensor("out", [S, H], F32, kind="ExternalOutput").ap()

    with tile.TileContext(nc) as tc:
        with ExitStack() as ctx:
            wp = ctx.enter_context(tc.tile_pool(name="weights", bufs=1))
            xp = ctx.enter_context(tc.tile_pool(name="acts", bufs=2))
            psp = ctx.enter_context(
                tc.tile_pool(name="psum", bufs=2, space="PSUM")
            )

            # ---- resident weights ----
            w1 = []
            for k in range(NHC):
                t_ = wp.tile([P, A], BF16, tag=f"w1_{k}")
                nc.sync.dma_start(t_[:], d_fc1T[k * P:(k + 1) * P, :])
                w1.append(t_)
            w2 = [[None] * NAC for _ in range(T)]
            for p in range(T):
                for ac in range(NAC):
                    t_ = wp.tile([P, H], BF16, tag=f"w2_{p}_{ac}")
                    nc.sync.dma_start(t_[:], d_W2T[p, ac * P:(ac + 1) * P, :])
                    w2[p][ac] = t_
            wmk = []
            for j in range(NHC):
                t_ = wp.tile([P, P], BF16, tag=f"wmk_{j}")
                nc.sync.dma_start(t_[:], d_Mk[j * P:(j + 1) * P, :])
                wmk.append(t_)
            wv = []
            for j in range(NHC):
                t_ = wp.tile([P, H], BF16, tag=f"wv_{j}")
                nc.sync.dma_start(t_[:], d_Wv[j * P:(j + 1) * P, :])
                wv.append(t_)
            b1 = wp.tile([P, NAC], F32, tag="b1")
            for ac in range(NAC):
                nc.sync.dma_start(b1[:, ac:ac + 1], d_fc1b[ac])
            b2 = wp.tile([P, NHC], F32, tag="b2")
            for hc in range(NHC):
                nc.sync.dma_start(b2[:, hc:hc + 1], d_fc2b[hc])
            ckt = wp.tile([P, 1], F32, tag="ck")
            nc.sync.dma_start(ckt[:], d_ck[:])
            g2t = wp.tile([P, NHC * T], F32, tag="g2")
            nc.sync.dma_start(g2t[:], d_g2[:])
            ident = wp.tile([P, P], BF16, tag="ident")
            make_identity(nc, ident[:])

            for st in range(NST):
                s0 = st * ST
                # ---- load xT chunks ----
                xt = []
                for k in range(NHC):
                    t_ = xp.tile([P, ST], BF16, name=f"xt{k}", tag=f"xt_{k}", bufs=1)
                    nc.sync.dma_start(t_[:], d_xT[k * P:(k + 1) * P, s0:s0 + ST])
                    xt.append(t_)
                # ---- fc1 -> h1T (gelu) ----
                h1 = []
                for ac in range(NAC):
                    ps = psp.tile([P, ST], F32, tag="ps_mm", bufs=3)
                    for k in range(NHC):
                        nc.tensor.matmul(
                            ps[:], w1[k][:, ac * P:(ac + 1) * P], xt[k][:],
                            start=(k == 0), stop=(k == NHC - 1),
                        )
                    t_ = xp.tile([P, ST], BF16, name=f"h1_{ac}", tag=f"h1_{ac}", bufs=2)
                    nc.scalar.activation(t_[:], ps[:], AF.Gelu, bias=b1[:, ac:ac + 1])
                    h1.append(t_)
                # ---- fc2 per task -> gated gelu store ----
                gst = [
                    xp.tile([P, T, ST], BF16, name=f"g{j}", tag=f"g_{j}", bufs=1)
                    for j in range(NHC)
                ]
                for p in range(T):
                    for j in range(NHC):
                        ps = psp.tile([P, ST], F32, tag="ps_mm", bufs=3)
                        for ac in range(NAC):
                            nc.tensor.matmul(
                                ps[:], w2[p][ac][:, j * P:(j + 1) * P], h1[ac][:],
                                start=(ac == 0), stop=(ac == NAC - 1),
                            )
                        nc.scalar.activation(
                            gst[j][:, p, :], ps[:], AF.Gelu, bias=b2[:, j:j + 1]
                        )
                        nc.vector.tensor_scalar_mul(
                            gst[j][:, p, :], gst[j][:, p, :],
                            g2t[:, j * T + p:j * T + p + 1],
                        )
                # ---- scores (batched: one ACT table switch) -> e = exp ----
                e_t = xp.tile([P, T, ST], F32, tag="e", bufs=1)
                for p in range(T):
                    ps_s = psp.tile([P, ST], F32, tag="ps_s", bufs=1)
                    for j in range(NHC):
                        nc.tensor.matmul(
                            ps_s[:], wmk[j][:], gst[j][:, p, :],
                            start=(j == 0), stop=(j == NHC - 1),
                        )
                    nc.scalar.activation(e_t[:, p, :], ps_s[:], AF.Exp, bias=ckt[:])
                # ---- softmax over tasks (no max-sub: |scores| < 1) ----
                d0 = xp.tile([P, ST], F32, tag="den", bufs=3)
                d1 = xp.tile([P, ST], F32, tag="den", bufs=3)
                d2 = xp.tile([P, ST], F32, tag="den", bufs=3)
                nc.vector.tensor_add(d0[:], e_t[:, 0, :], e_t[:, 1, :])
                nc.vector.tensor_add(d1[:], e_t[:, 2, :], e_t[:, 3, :])
                nc.vector.tensor_add(d2[:], e_t[:, 4, :], e_t[:, 5, :])
                nc.vector.tensor_add(d0[:], d0[:], d1[:])
                nc.vector.tensor_add(d0[:], d0[:], d2[:])
                nc.vector.reciprocal(d0[:], d0[:])
                for p in range(T):
                    nc.vector.tensor_mul(e_t[:, p, :], e_t[:, p, :], d0[:])
                # ---- V GEMM + probs-weighted task sum ----
                ctxs = []
                for hc in range(NHC):
                    eng = nc.vector if hc % 2 == 0 else nc.gpsimd
                    sc = []
                    for p in range(T):
                        ps_v = psp.tile([P, ST], F32, tag="ps_v")
                        for j in range(NHC):
                            nc.tensor.matmul(
                                ps_v[:], wv[j][:, hc * P:(hc + 1) * P],
                                gst[j][:, p, :],
                                start=(j == 0), stop=(j == NHC - 1),
                            )
                        t_ = xp.tile([P, ST], BF16, name=f"sc{p}", tag="sc", bufs=8)
                        nc.vector.tensor_mul(t_[:], ps_v[:], e_t[:, p, :])
                        sc.append(t_)
                    eng.tensor_add(sc[0][:], sc[0][:], sc[1][:])
                    eng.tensor_add(sc[2][:], sc[2][:], sc[3][:])
                    eng.tensor_add(sc[4][:], sc[4][:], sc[5][:])
                    eng.tensor_add(sc[0][:], sc[0][:], sc[2][:])
                    cx = xp.tile([P, ST], BF16, tag="ctx", bufs=10)
                    eng.tensor_add(cx[:], sc[0][:], sc[4][:])
                    ctxs.append(cx)
                # ---- transpose + head-permute + residual + store ----
                for sb in range(ST // P):
                    r0 = s0 + sb * P
                    xr = xp.tile([P, H], F32, tag="xr", bufs=2)
                    nc.sync.dma_start(xr[:], d_xres[r0:r0 + P, :])
                    ot = xp.tile([P, H], F32, tag="ot", bufs=2)
                    for h2 in range(2):
                        ps_t = psp.tile([P, ST], BF16, tag="ps_t")
                        for q in range(4):
                            hc = h2 * 4 + q
                            nc.tensor.transpose(
                                ps_t[:, q * P:(q + 1) * P],
                                ctxs[hc][:, sb * P:(sb + 1) * P],
                                ident[:],
                            )
                        # out cols h' = d*16 + h2*8 + c*2 + nl  for psum col (c,nl,d)
                        o_ap = ot[:].rearrange(
                            "p (d h2 c nl) -> p h2 c nl d", d=HD, h2=2, c=4, nl=2
                        )[:, h2]
                        x_ap = xr[:].rearrange(
                            "p (d h2 c nl) -> p h2 c nl d", d=HD, h2=2, c=4, nl=2
                        )[:, h2]
                        p_ap = ps_t[:].rearrange("p (c nl d) -> p c nl d", c=4, nl=2, d=HD)
                        nc.vector.tensor_add(o_ap, p_ap, x_ap)
                    nc.sync.dma_start(d_out[r0:r0 + P, :], ot[:])
    nc.compile()
    return nc


def _sigmoid(x):
    with np.errstate(over="ignore"):
        return 1.0 / (1.0 + np.exp(-x))


def _host_prep(x, fc1_w, fc1_b, fc2_w, fc2_b, efc1, efc2, etask,
               q_w, q_b, k_w, k_b, v_w, v_b, equery, ekey, evalue, t, s):
    f64 = np.float64
    t = int(t)
    s = float(s)
    assert t + 1 == T and x.shape == (B, S, H)
    fc1_w = np.asarray(fc1_w, f64); fc1_b = np.asarray(fc1_b, f64)
    fc2_w = np.asarray(fc2_w, f64); fc2_b = np.asarray(fc2_b, f64)
    efc1 = np.asarray(efc1, f64); efc2 = np.asarray(efc2, f64)
    etask = np.asarray(etask, f64)
    q_w = np.asarray(q_w, f64); q_b = np.asarray(q_b, f64)
    k_w = np.asarray(k_w, f64); k_b = np.asarray(k_b, f64)
    v_w = np.asarray(v_w, f64); v_b = np.asarray(v_b, f64)
    equery = np.asarray(equery, f64); ekey = np.asarray(ekey, f64)
    evalue = np.asarray(evalue, f64)

    g1 = np.stack([_sigmoid(s * efc1[t])] + [_sigmoid(SMAX * efc1[p]) for p in range(t)])
    g2 = np.stack([_sigmoid(s * efc2[t])] + [_sigmoid(SMAX * efc2[p]) for p in range(t)])
    gq = _sigmoid(s * equery[t]); gk = _sigmoid(s * ekey[t]); gv = _sigmoid(s * evalue[t])

    q_vec = (etask[t] @ q_w.T + q_b) * gq
    q_mat = q_vec.reshape(NH, HD)
    kwg = k_w * gk[:, None]
    Mk = np.einsum("nd,ndj->dj", q_mat, kwg.reshape(NH, HD, H)) / np.sqrt(HD)
    ck = np.einsum("nd,nd->d", q_mat, (k_b * gk).reshape(NH, HD)) / np.sqrt(HD)

    MkTdup = np.ascontiguousarray(
        np.concatenate([Mk.T, Mk.T], axis=1).astype(NPBF16))       # [H,128]
    ck_dup = np.tile(ck, 2).astype(np.float32).reshape(P, 1)
    W2T = np.ascontiguousarray(
        (fc2_w.T[None] * g1[:, :, None]).astype(NPBF16))           # [T,A,H]
    WvT = np.ascontiguousarray((v_w * gv[:, None]).T.astype(NPBF16))  # [H,H]
    vbg_perm = (v_b * gv).reshape(NH, HD).T.reshape(H)             # h' = d*16+n
    fc1T = np.ascontiguousarray(fc1_w.T.astype(NPBF16))            # [H,A]
    fc1b = fc1_b.astype(np.float32).reshape(NAC, P, 1)
    fc2b = fc2_b.astype(np.float32).reshape(NHC, P, 1)
    # g2sb[r, j*T+p] = g2[p, j*128+r]
    g2sb = np.ascontiguousarray(
        g2.reshape(T, NHC, P).transpose(2, 1, 0).reshape(P, NHC * T).astype(np.float32))

    shared = dict(fc1T=fc1T, fc1b=fc1b, W2T=W2T, fc2b=fc2b,
                  MkT=MkTdup, ck=ck_dup, WvT=WvT, g2sb=g2sb)
    per_core = []
    x32 = np.asarray(x, np.float32)
    xres_all = x32 + vbg_perm.astype(np.float32)[None, None, :]
    for b_ in range(B):
        m = dict(shared)
        m["xT"] = np.ascontiguousarray(x32[b_].T.astype(NPBF16))
        m["xres"] = np.ascontiguousarray(xres_all[b_])
        per_core.append(m)
    return per_core


def kernel(**inputs):
    if "nc" not in _CACHE:
        _CACHE["nc"] = _build_nc()
    nc = _CACHE["nc"]
    in_maps = _host_prep(**inputs)
    res = run_bass_kernel_spmd(nc, in_maps, core_ids=list(range(B)))
    out = np.stack([res.results[c]["out"] for c in range(B)], axis=0)
    return out.astype(np.float32)


# revision 10
# speedup vs baseline: 2.7495x; 1.0041x over previous
"""Trainium2 Bass kernel for nn_BertAdapterAttentionMask.

Math restructuring (validated vs reference in fp64):
  * The query comes from a broadcast task embedding -> q is one [H] vector,
    constant over (b, s). The K projection therefore collapses to a rank-hd
    GEMM:  scores[., d] = hid @ Mk[d, :] + ck[d], Mk = fold(q, gk, k_w)/sqrt(hd).
  * Per-task adapter gates fold into fc2 weights (host side).
  * softmax(scores) sums to 1 over tasks, so the V bias contributes a constant
    vector; it is folded into the residual x on the host.
  * scores GEMM uses a column-duplicated Mk (M=128) so probs come out
    partition-duplicated for free (broadcast over the 2x64 row halves).

Per-core layout: fully "transposed" pipeline (features on partitions,
sequence on the free axis). Data-parallel over batch B=8 across 8 cores.
All GEMM operands bf16 (1 cyc/row on PE), fp32 accumulation + softmax.
"""

import numpy as np
import ml_dtypes
from contextlib import ExitStack

import concourse.bass as bass
import concourse.tile as tile
from concourse import bacc, mybir
from concourse.bass_utils import run_bass_kernel_spmd
from concourse.masks import make_identity

AF = mybir.ActivationFunctionType
BF16 = mybir.dt.bfloat16
F32 = mybir.dt.float32
NPBF16 = ml_dtypes.bfloat16

B, S, H, A, NH, HD = 8, 2048, 1024, 512, 16, 64
T = 6              # tasks = t + 1
P = 128
ST = 512           # s-tile (free-dim tile)
NST = S // ST      # 4
NHC = H // P       # 8 h-chunks
NAC = A // P       # 4 a-chunks
SMAX = 400.0

_CACHE = {}


def _build_nc():
    nc = bacc.Bacc("TRN2", target_bir_lowering=False, debug=False)

    d_xT = nc.dram_tensor("xT", [H, S], BF16, kind="ExternalInput").ap()
    d_xres = nc.dram_tensor("xres", [S, H], F32, kind="ExternalInput").ap()
    d_fc1T = nc.dram_tensor("fc1T", [H, A], BF16, kind="ExternalInput").ap()
    d_fc1b = nc.dram_tensor("fc1b", [NAC, P, 1], F32, kind="ExternalInput").ap()
    d_W2T = nc.dram_tensor("W2T", [T, A, H], BF16, kind="ExternalInput").ap()
    d_fc2b = nc.dram_tensor("fc2b", [NHC, P, 1], F32, kind="ExternalInput").ap()
    d_Mk = nc.dram_tensor("MkT", [H, P], BF16, kind="ExternalInput").ap()
    d_ck = nc.dram_tensor("ck", [P, 1], F32, kind="ExternalInput").ap()
    d_Wv = nc.dram_tensor("WvT", [H, H], BF16, kind="ExternalInput").ap()
    d_g2 = nc.dram_tensor("g2sb", [P, NHC * T], F32, kind="ExternalInput").ap()
    d_out = nc.dram_tensor("out", [S, H], F32, kind="ExternalOutput").ap()

    with tile.TileContext(nc) as tc:
        with ExitStack() as ctx:
            wp = ctx.enter_context(tc.tile_pool(name="weights", bufs=1))
            xp = ctx.enter_context(tc.tile_pool(name="acts", bufs=2))
            psp = ctx.enter_context(
                tc.tile_pool(name="psum", bufs=2, space="PSUM")
            )

            # ---- resident weights ----
            w1 = []
            for k in range(NHC):
                t_ = wp.tile([P, A], BF16, tag=f"w1_{k}")
                nc.sync.dma_start(t_[:], d_fc1T[k * P:(k + 1) * P, :])
                w1.append(t_)
            w2 = [[None] * NAC for _ in range(T)]
            for p in range(T):
                for ac in range(NAC):
                    t_ = wp.tile([P, H], BF16, tag=f"w2_{p}_{ac}")
                    nc.sync.dma_start(t_[:], d_W2T[p, ac * P:(ac + 1) * P, :])
                    w2[p][ac] = t_
            wmk = []
            for j in range(NHC):
                t_ = wp.tile([P, P], BF16, tag=f"wmk_{j}")
                nc.sync.dma_start(t_[:], d_Mk[j * P:(j + 1) * P, :])
                wmk.append(t_)
            wv = []
            for j in range(NHC):
                t_ = wp.tile([P, H], BF16, tag=f"wv_{j}")
                nc.sync.dma_start(t_[:], d_Wv[j * P:(j + 1) * P, :])
                wv.append(t_)
            b1 = wp.tile([P, NAC], F32, tag="b1")
            for ac in range(NAC):
                nc.sync.dma_start(b1[:, ac:ac + 1], d_fc1b[ac])
            b2 = wp.tile([P, NHC], F32, tag="b2")
            for hc in range(NHC):
                nc.sync.dma_start(b2[:, hc:hc + 1], d_fc2b[hc])
            ckt = wp.tile([P, 1], F32, tag="ck")
            nc.sync.dma_start(ckt[:], d_ck[:])
            g2t = wp.tile([P, NHC * T], F32, tag="g2")
            nc.sync.dma_start(g2t[:], d_g2[:])
            ident = wp.tile([P, P], BF16, tag="ident")
            make_identity(nc, ident[:])

            for st in range(NST):
                s0 = st * ST
                # ---- load xT chunks ----
                xt = []
                for k in range(NHC):
                    t_ = xp.tile([P, ST], BF16, name=f"xt{k}", tag=f"xt_{k}", bufs=1)
                    nc.sync.dma_start(t_[:], d_xT[k * P:(k + 1) * P, s0:s0 + ST])
                    xt.append(t_)
                # ---- fc1 -> h1T (gelu) ----
                h1 = []
                for ac in range(NAC):
                    ps = psp.tile([P, ST], F32, tag="ps_mm", bufs=3)
                    for k in range(NHC):
                        nc.tensor.matmul(
                            ps[:], w1[k][:, ac * P:(ac + 1) * P], xt[k][:],
                            start=(k == 0), stop=(k == NHC - 1),
                        )
                    t_ = xp.tile([P, ST], BF16, name=f"h1_{ac}", tag=f"h1_{ac}", bufs=2)
                    nc.scalar.activation(t_[:], ps[:], AF.Gelu, bias=b1[:, ac:ac + 1])
                    h1.append(t_)
                # ---- fc2 per task -> gated gelu store ----
                gst = [
                    xp.tile([P, T, ST], BF16, name=f"g{j}", tag=f"g_{j}", bufs=1)
                    for j in range(NHC)
                ]
                for p in range(T):
                    for j in range(NHC):
                        ps = psp.tile([P, ST], F32, tag="ps_mm", bufs=3)
                        for ac in range(NAC):
                            nc.tensor.matmul(
                                ps[:], w2[p][ac][:, j * P:(j + 1) * P], h1[ac][:],
                                start=(ac == 0), stop=(ac == NAC - 1),
                            )
                        nc.scalar.activation(
                            gst[j][:, p, :], ps[:], AF.Gelu, bias=b2[:, j:j + 1]
                        )
                        nc.vector.tensor_scalar_mul(
                            gst[j][:, p, :], gst[j][:, p, :],
                            g2t[:, j * T + p:j * T + p + 1],
                        )
                # ---- scores (batched: one ACT table switch) -> e = exp ----
                e_t = xp.tile([P, T, ST], F32, tag="e", bufs=1)
                for p in range(T):
                    ps_s = psp.tile([P, ST], F32, tag="ps_s", bufs=1)
                    for j in range(NHC):
                        nc.tensor.matmul(
                            ps_s[:], wmk[j][:], gst[j][:, p, :],
                            start=(j == 0), stop=(j == NHC - 1),
                        )
                    nc.scalar.activation(e_t[:, p, :], ps_s[:], AF.Exp, bias=ckt[:])
                # ---- softmax over tasks (no max-sub: |scores| < 1) ----
                d0 = xp.tile([P, ST], F32, tag="den", bufs=3)
                d1 = xp.tile([P, ST], F32, tag="den", bufs=3)
                d2 = xp.tile([P, ST], F32, tag="den", bufs=3)
                nc.vector.tensor_add(d0[:], e_t[:, 0, :], e_t[:, 1, :])
                nc.vector.tensor_add(d1[:], e_t[:, 2, :], e_t[:, 3, :])
                nc.vector.tensor_add(d2[:], e_t[:, 4, :], e_t[:, 5, :])
                nc.vector.tensor_add(d0[:], d0[:], d1[:])
                nc.vector.tensor_add(d0[:], d0[:], d2[:])
                nc.vector.reciprocal(d0[:], d0[:])
                for p in range(T):
                    nc.vector.tensor_mul(e_t[:, p, :], e_t[:, p, :], d0[:])
                # ---- V GEMM + probs-weighted task sum + transpose/store ----
                # phase E (transpose+residual) is done per 4-chunk half so it
                # overlaps the other half's V GEMMs on the PE.
                xrs, ots = [], []
                for sb in range(ST // P):
                    r0 = s0 + sb * P
                    xr = xp.tile([P, H], F32, name=f"xr{sb}", tag=f"xr_{sb}", bufs=1)
                    nc.sync.dma_start(xr[:], d_xres[r0:r0 + P, :])
                    ot = xp.tile([P, H], F32, name=f"ot{sb}", tag=f"ot_{sb}", bufs=1)
                    xrs.append(xr)
                    ots.append(ot)
                for h2 in range(2):
                    ctxs = []
                    for q in range(4):
                        hc = h2 * 4 + q
                        eng = nc.gpsimd if q < 2 else nc.vector
                        sc = []
                        for p in range(T):
                            ps_v = psp.tile([P, ST], F32, tag="ps_v")
                            for j in range(NHC):
                                nc.tensor.matmul(
                                    ps_v[:], wv[j][:, hc * P:(hc + 1) * P],
                                    gst[j][:, p, :],
                                    start=(j == 0), stop=(j == NHC - 1),
                                )
                            t_ = xp.tile([P, ST], BF16, name=f"sc{p}", tag="sc", bufs=8)
                            nc.vector.tensor_mul(t_[:], ps_v[:], e_t[:, p, :])
                            sc.append(t_)
                        eng.tensor_add(sc[0][:], sc[0][:], sc[1][:])
                        eng.tensor_add(sc[2][:], sc[2][:], sc[3][:])
                        eng.tensor_add(sc[4][:], sc[4][:], sc[5][:])
                        eng.tensor_add(sc[0][:], sc[0][:], sc[2][:])
                        cx = xp.tile([P, ST], BF16, tag="ctx", bufs=6)
                        eng.tensor_add(cx[:], sc[0][:], sc[4][:])
                        ctxs.append(cx)
                    for sb in range(ST // P):
                        ps_t = psp.tile([P, ST], BF16, tag="ps_t")
                        for q in range(4):
                            nc.tensor.transpose(
                                ps_t[:, q * P:(q + 1) * P],
                                ctxs[q][:, sb * P:(sb + 1) * P],
                                ident[:],
                            )
                        # out cols h' = d*16 + h2*8 + c*2 + nl  for psum col (c,nl,d)
                        o_ap = ots[sb][:].rearrange(
                            "p (d h2 c nl) -> p h2 c nl d", d=HD, h2=2, c=4, nl=2
                        )[:, h2]
                        x_ap = xrs[sb][:].rearrange(
                            "p (d h2 c nl) -> p h2 c nl d", d=HD, h2=2, c=4, nl=2
                        )[:, h2]
                        p_ap = ps_t[:].rearrange("p (c nl d) -> p c nl d", c=4, nl=2, d=HD)
                        nc.vector.tensor_add(o_ap, p_ap, x_ap)
                for sb in range(ST // P):
                    nc.sync.dma_start(d_out[s0 + sb * P:s0 + (sb + 1) * P, :], ots[sb][:])
    nc.compile()
    return nc


def _sigmoid(x):
    with np.errstate(over="ignore"):
        return 1.0 / (1.0 + np.exp(-x))


def _host_prep(x, fc1_w, fc1_b, fc2_w, fc2_b, efc1, efc2, etask,
               q_w, q_b, k_w, k_b, v_w, v_b, equery, ekey, evalue, t, s):
    f64 = np.float64
    t = int(t)
    s = float(s)
    assert t + 1 == T and x.shape == (B, S, H)
    fc1_w = np.asarray(fc1_w, f64); fc1_b = np.asarray(fc1_b, f64)
    fc2_w = np.asarray(fc2_w, f64); fc2_b = np.asarray(fc2_b, f64)
    efc1 = np.asarray(efc1, f64); efc2 = np.asarray(efc2, f64)
    etask = np.asarray(etask, f64)
    q_w = np.asarray(q_w, f64); q_b = np.asarray(q_b, f64)
    k_w = np.asarray(k_w, f64); k_b = np.asarray(k_b, f64)
    v_w = np.asarray(v_w, f64); v_b = np.asarray(v_b, f64)
    equery = np.asarray(equery, f64); ekey = np.asarray(ekey, f64)
    evalue = np.asarray(evalue, f64)

    g1 = np.stack([_sigmoid(s * efc1[t])] + [_sigmoid(SMAX * efc1[p]) for p in range(t)])
    g2 = np.stack([_sigmoid(s * efc2[t])] + [_sigmoid(SMAX * efc2[p]) for p in range(t)])
    gq = _sigmoid(s * equery[t]); gk = _sigmoid(s * ekey[t]); gv = _sigmoid(s * evalue[t])

    q_vec = (etask[t] @ q_w.T + q_b) * gq
    q_mat = q_vec.reshape(NH, HD)
    kwg = k_w * gk[:, None]
    Mk = np.einsum("nd,ndj->dj", q_mat, kwg.reshape(NH, HD, H)) / np.sqrt(HD)
    ck = np.einsum("nd,nd->d", q_mat, (k_b * gk).reshape(NH, HD)) / np.sqrt(HD)

    MkTdup = np.ascontiguousarray(
        np.concatenate([Mk.T, Mk.T], axis=1).astype(NPBF16))       # [H,128]
    ck_dup = np.tile(ck, 2).astype(np.float32).reshape(P, 1)
    W2T = np.ascontiguousarray(
        (fc2_w.T[None] * g1[:, :, None]).astype(NPBF16))           # [T,A,H]
    WvT = np.ascontiguousarray((v_w * gv[:, None]).T.astype(NPBF16))  # [H,H]
    vbg_perm = (v_b * gv).reshape(NH, HD).T.reshape(H)             # h' = d*16+n
    fc1T = np.ascontiguousarray(fc1_w.T.astype(NPBF16))            # [H,A]
    fc1b = fc1_b.astype(np.float32).reshape(NAC, P, 1)
    fc2b = fc2_b.astype(np.float32).reshape(NHC, P, 1)
    # g2sb[r, j*T+p] = g2[p, j*128+r]
    g2sb = np.ascontiguousarray(
        g2.reshape(T, NHC, P).transpose(2, 1, 0).reshape(P, NHC * T).astype(np.float32))

    shared = dict(fc1T=fc1T, fc1b=fc1b, W2T=W2T, fc2b=fc2b,
                  MkT=MkTdup, ck=ck_dup, WvT=WvT, g2sb=g2sb)
    per_core = []
    x32 = np.asarray(x, np.float32)
    xres_all = x32 + vbg_perm.astype(np.float32)[None, None, :]
    for b_ in range(B):
        m = dict(shared)
        m["xT"] = np.ascontiguousarray(x32[b_].T.astype(NPBF16))
        m["xres"] = np.ascontiguousarray(xres_all[b_])
        per_core.append(m)
    return per_core


def kernel(**inputs):
    if "nc" not in _CACHE:
        _CACHE["nc"] = _build_nc()
    nc = _CACHE["nc"]
    in_maps = _host_prep(**inputs)
    last_err = None
    for _attempt in range(3):
        try:
            res = run_bass_kernel_spmd(nc, in_maps, core_ids=list(range(B)))
            break
        except Exception as e:  # transient NRT device errors: retry
            last_err = e
    else:
        raise last_err
    out = np.stack([res.results[c]["out"] for c in range(B)], axis=0)
    return out.astype(np.float32)


# revision 13
# speedup vs baseline: 2.8618x; 1.0408x over previous
"""Trainium2 Bass kernel for nn_BertAdapterAttentionMask.

Math restructuring (validated vs reference in fp64):
  * The query comes from a broadcast task embedding -> q is one [H] vector,
    constant over (b, s). The K projection therefore collapses to a rank-hd
    GEMM:  scores[., d] = hid @ Mk[d, :] + ck[d], Mk = fold(q, gk, k_w)/sqrt(hd).
  * Per-task adapter gates fold into fc2 weights (host side).
  * softmax(scores) sums to 1 over tasks, so the V bias contributes a constant
    vector; it is folded into the residual x on the host.
  * scores GEMM uses a column-duplicated Mk (M=128) so probs come out
    partition-duplicated for free (broadcast over the 2x64 row halves).

Per-core layout: fully "transposed" pipeline (features on partitions,
sequence on the free axis). Data-parallel over batch B=8 across 8 cores.
All GEMM operands bf16 (1 cyc/row on PE), fp32 accumulation + softmax.
"""

import numpy as np
import ml_dtypes
from contextlib import ExitStack

import concourse.bass as bass
import concourse.tile as tile
from concourse import bacc, mybir
from concourse.bass_utils import run_bass_kernel_spmd
from concourse.masks import make_identity

AF = mybir.ActivationFunctionType
BF16 = mybir.dt.bfloat16
F32 = mybir.dt.float32
NPBF16 = ml_dtypes.bfloat16

B, S, H, A, NH, HD = 8, 2048, 1024, 512, 16, 64
T = 6              # tasks = t + 1
P = 128
ST = 512           # s-tile (free-dim tile)
NST = S // ST      # 4
NHC = H // P       # 8 h-chunks
NAC = A // P       # 4 a-chunks
SMAX = 400.0

_CACHE = {}


def _build_nc():
    nc = bacc.Bacc("TRN2", target_bir_lowering=False, debug=False)

    d_xT = nc.dram_tensor("xT", [H, S], BF16, kind="ExternalInput").ap()
    d_xres = nc.dram_tensor("xres", [S, H], F32, kind="ExternalInput").ap()
    d_fc1T = nc.dram_tensor("fc1T", [H, A], BF16, kind="ExternalInput").ap()
    d_fc1b = nc.dram_tensor("fc1b", [NAC, P, 1], F32, kind="ExternalInput").ap()
    d_W2T = nc.dram_tensor("W2T", [T, A, H], BF16, kind="ExternalInput").ap()
    d_fc2b = nc.dram_tensor("fc2b", [NHC, P, 1], F32, kind="ExternalInput").ap()
    d_Mk = nc.dram_tensor("MkT", [H, P], BF16, kind="ExternalInput").ap()
    d_ck = nc.dram_tensor("ck", [P, 1], F32, kind="ExternalInput").ap()
    d_Wv = nc.dram_tensor("WvT", [H, H], BF16, kind="ExternalInput").ap()
    d_g2 = nc.dram_tensor("g2sb", [P, NHC * T], F32, kind="ExternalInput").ap()
    d_out = nc.dram_tensor("out", [S, H], F32, kind="ExternalOutput").ap()

    with tile.TileContext(nc) as tc:
        with ExitStack() as ctx:
            wp = ctx.enter_context(tc.tile_pool(name="weights", bufs=1))
            xp = ctx.enter_context(tc.tile_pool(name="acts", bufs=2))
            psp = ctx.enter_context(
                tc.tile_pool(name="psum", bufs=2, space="PSUM")
            )

            # ---- resident weights (DMA order = first-use order: fc1 deps
            # first so the PE can start within a few us) ----
            w1 = []
            for k in range(NHC):
                t_ = wp.tile([P, A], BF16, tag=f"w1_{k}")
                nc.sync.dma_start(t_[:], d_fc1T[k * P:(k + 1) * P, :])
                w1.append(t_)
            b1 = wp.tile([P, NAC], F32, tag="b1")
            for ac in range(NAC):
                nc.sync.dma_start(b1[:, ac:ac + 1], d_fc1b[ac])
            xt0 = []
            for k in range(NHC):
                t_ = xp.tile([P, ST], BF16, name=f"xt{k}", tag=f"xt_{k}", bufs=1)
                nc.sync.dma_start(t_[:], d_xT[k * P:(k + 1) * P, 0:ST])
                xt0.append(t_)
            w2 = [[None] * NAC for _ in range(T)]
            for p in range(T):
                for ac in range(NAC):
                    t_ = wp.tile([P, H], BF16, tag=f"w2_{p}_{ac}")
                    nc.sync.dma_start(t_[:], d_W2T[p, ac * P:(ac + 1) * P, :])
                    w2[p][ac] = t_
            b2 = wp.tile([P, NHC], F32, tag="b2")
            for hc in range(NHC):
                nc.sync.dma_start(b2[:, hc:hc + 1], d_fc2b[hc])
            g2t = wp.tile([P, NHC * T], F32, tag="g2")
            nc.sync.dma_start(g2t[:], d_g2[:])
            wmk = []
            for j in range(NHC):
                t_ = wp.tile([P, P], BF16, tag=f"wmk_{j}")
                nc.sync.dma_start(t_[:], d_Mk[j * P:(j + 1) * P, :])
                wmk.append(t_)
            ckt = wp.tile([P, 1], F32, tag="ck")
            nc.sync.dma_start(ckt[:], d_ck[:])
            wv = []
            for j in range(NHC):
                t_ = wp.tile([P, H], BF16, tag=f"wv_{j}")
                nc.sync.dma_start(t_[:], d_Wv[j * P:(j + 1) * P, :])
                wv.append(t_)
            ident = wp.tile([P, P], BF16, tag="ident")
            make_identity(nc, ident[:])

            pending_E = []   # deferred phase-E emitters (overlap next fc1)
            for st in range(NST):
                s0 = st * ST
                # ---- load xT chunks ----
                if st == 0:
                    xt = xt0
                else:
                    xt = []
                    for k in range(NHC):
                        t_ = xp.tile([P, ST], BF16, name=f"xt{k}", tag=f"xt_{k}", bufs=1)
                        nc.sync.dma_start(t_[:], d_xT[k * P:(k + 1) * P, s0:s0 + ST])
                        xt.append(t_)
                # ---- fc1 -> h1T (gelu) ----
                h1 = []
                for ac in range(NAC):
                    ps = psp.tile([P, ST], F32, tag="ps_mm", bufs=3)
                    for k in range(NHC):
                        nc.tensor.matmul(
                            ps[:], w1[k][:, ac * P:(ac + 1) * P], xt[k][:],
                            start=(k == 0), stop=(k == NHC - 1),
                        )
                    t_ = xp.tile([P, ST], BF16, name=f"h1_{ac}", tag=f"h1_{ac}", bufs=2)
                    nc.scalar.activation(t_[:], ps[:], AF.Gelu, bias=b1[:, ac:ac + 1])
                    h1.append(t_)
                # flush previous s-tile's deferred phase-E (overlaps fc2 GEMMs)
                for fn in pending_E:
                    fn()
                pending_E = []
                # ---- fc2 per task -> gated gelu store ----
                gst = [
                    xp.tile([P, T, ST], BF16, name=f"g{j}", tag=f"g_{j}", bufs=1)
                    for j in range(NHC)
                ]
                for p in range(T):
                    for j in range(NHC):
                        ps = psp.tile([P, ST], F32, tag="ps_mm", bufs=3)
                        for ac in range(NAC):
                            nc.tensor.matmul(
                                ps[:], w2[p][ac][:, j * P:(j + 1) * P], h1[ac][:],
                                start=(ac == 0), stop=(ac == NAC - 1),
                            )
                        nc.scalar.activation(
                            gst[j][:, p, :], ps[:], AF.Gelu, bias=b2[:, j:j + 1]
                        )
                        nc.vector.tensor_scalar_mul(
                            gst[j][:, p, :], gst[j][:, p, :],
                            g2t[:, j * T + p:j * T + p + 1],
                        )
                # ---- scores (batched: one ACT table switch) -> e = exp ----
                e_t = xp.tile([P, T, ST], F32, tag="e", bufs=1)
                for p in range(T):
                    ps_s = psp.tile([P, ST], F32, tag="ps_s", bufs=1)
                    for j in range(NHC):
                        nc.tensor.matmul(
                            ps_s[:], wmk[j][:], gst[j][:, p, :],
                            start=(j == 0), stop=(j == NHC - 1),
                        )
                    nc.scalar.activation(e_t[:, p, :], ps_s[:], AF.Exp, bias=ckt[:])
                # ---- softmax over tasks (no max-sub: |scores| < 1) ----
                d0 = xp.tile([P, ST], F32, tag="den", bufs=3)
                d1 = xp.tile([P, ST], F32, tag="den", bufs=3)
                d2 = xp.tile([P, ST], F32, tag="den", bufs=3)
                nc.vector.tensor_add(d0[:], e_t[:, 0, :], e_t[:, 1, :])
                nc.vector.tensor_add(d1[:], e_t[:, 2, :], e_t[:, 3, :])
                nc.vector.tensor_add(d2[:], e_t[:, 4, :], e_t[:, 5, :])
                nc.vector.tensor_add(d0[:], d0[:], d1[:])
                nc.vector.tensor_add(d0[:], d0[:], d2[:])
                nc.vector.reciprocal(d0[:], d0[:])
                for p in range(T):
                    nc.vector.tensor_mul(e_t[:, p, :], e_t[:, p, :], d0[:])
                # ---- V GEMM + probs-weighted task sum + transpose/store ----
                # phase E (transpose + head-permute + residual) for each
                # 4-chunk half is emitted late so the PE overlaps it with
                # later GEMM work: half1-E after half2's V MMs, half2-E after
                # the NEXT s-tile's fc1 (via pending_E).
                xrs, ots = [], []
                for sb in range(ST // P):
                    r0 = s0 + sb * P
                    xr = xp.tile([P, H], F32, name=f"xr{sb}", tag=f"xr_{sb}", bufs=1)
                    nc.sync.dma_start(xr[:], d_xres[r0:r0 + P, :])
                    ot = xp.tile([P, H], F32, name=f"ot{sb}", tag=f"ot_{sb}", bufs=1)
                    xrs.append(xr)
                    ots.append(ot)

                def emit_E(ctxs, h2, ots=ots, xrs=xrs, s0=s0, last=False):
                    for sb in range(ST // P):
                        ps_t = psp.tile([P, ST], BF16, tag="ps_t", name="ps_t")
                        for q in range(4):
                            nc.tensor.transpose(
                                ps_t[:, q * P:(q + 1) * P],
                                ctxs[q][:, sb * P:(sb + 1) * P],
                                ident[:],
                            )
                        # out cols h' = d*16 + h2*8 + c*2 + nl for psum (c,nl,d)
                        o_ap = ots[sb][:].rearrange(
                            "p (d h2 c nl) -> p h2 c nl d", d=HD, h2=2, c=4, nl=2
                        )[:, h2]
                        x_ap = xrs[sb][:].rearrange(
                            "p (d h2 c nl) -> p h2 c nl d", d=HD, h2=2, c=4, nl=2
                        )[:, h2]
                        p_ap = ps_t[:].rearrange("p (c nl d) -> p c nl d", c=4, nl=2, d=HD)
                        nc.vector.tensor_add(o_ap, p_ap, x_ap)
                        if last:
                            nc.sync.dma_start(
                                d_out[s0 + sb * P:s0 + (sb + 1) * P, :], ots[sb][:]
                            )

                halves = []
                for h2 in range(2):
                    ctxs = []
                    for q in range(4):
                        hc = h2 * 4 + q
                        eng = nc.gpsimd if q < 2 else nc.vector
                        sc = []
                        for p in range(T):
                            ps_v = psp.tile([P, ST], F32, tag="ps_v")
                            for j in range(NHC):
                                nc.tensor.matmul(
                                    ps_v[:], wv[j][:, hc * P:(hc + 1) * P],
                                    gst[j][:, p, :],
                                    start=(j == 0), stop=(j == NHC - 1),
                                )
                            t_ = xp.tile([P, ST], BF16, name=f"sc{p}", tag="sc", bufs=8)
                            nc.vector.tensor_mul(t_[:], ps_v[:], e_t[:, p, :])
                            sc.append(t_)
                        eng.tensor_add(sc[0][:], sc[0][:], sc[1][:])
                        eng.tensor_add(sc[2][:], sc[2][:], sc[3][:])
                        eng.tensor_add(sc[4][:], sc[4][:], sc[5][:])
                        eng.tensor_add(sc[0][:], sc[0][:], sc[2][:])
                        cx = xp.tile([P, ST], BF16, tag="ctx", bufs=10)
                        eng.tensor_add(cx[:], sc[0][:], sc[4][:])
                        ctxs.append(cx)
                    halves.append(ctxs)
                emit_E(halves[0], 0)
                pending_E.append(lambda e=emit_E, c=halves[1]: e(c, 1, last=True))
            for fn in pending_E:
                fn()
            pending_E = []
    nc.compile()
    return nc


def _sigmoid(x):
    with np.errstate(over="ignore"):
        return 1.0 / (1.0 + np.exp(-x))


def _host_prep(x, fc1_w, fc1_b, fc2_w, fc2_b, efc1, efc2, etask,
               q_w, q_b, k_w, k_b, v_w, v_b, equery, ekey, evalue, t, s):
    f64 = np.float64
    t = int(t)
    s = float(s)
    assert t + 1 == T and x.shape == (B, S, H)
    fc1_w = np.asarray(fc1_w, f64); fc1_b = np.asarray(fc1_b, f64)
    fc2_w = np.asarray(fc2_w, f64); fc2_b = np.asarray(fc2_b, f64)
    efc1 = np.asarray(efc1, f64); efc2 = np.asarray(efc2, f64)
    etask = np.asarray(etask, f64)
    q_w = np.asarray(q_w, f64); q_b = np.asarray(q_b, f64)
    k_w = np.asarray(k_w, f64); k_b = np.asarray(k_b, f64)
    v_w = np.asarray(v_w, f64); v_b = np.asarray(v_b, f64)
    equery = np.asarray(equery, f64); ekey = np.asarray(ekey, f64)
    evalue = np.asarray(evalue, f64)

    g1 = np.stack([_sigmoid(s * efc1[t])] + [_sigmoid(SMAX * efc1[p]) for p in range(t)])
    g2 = np.stack([_sigmoid(s * efc2[t])] + [_sigmoid(SMAX * efc2[p]) for p in range(t)])
    gq = _sigmoid(s * equery[t]); gk = _sigmoid(s * ekey[t]); gv = _sigmoid(s * evalue[t])

    q_vec = (etask[t] @ q_w.T + q_b) * gq
    q_mat = q_vec.reshape(NH, HD)
    kwg = k_w * gk[:, None]
    Mk = np.einsum("nd,ndj->dj", q_mat, kwg.reshape(NH, HD, H)) / np.sqrt(HD)
    ck = np.einsum("nd,nd->d", q_mat, (k_b * gk).reshape(NH, HD)) / np.sqrt(HD)

    MkTdup = np.ascontiguousarray(
        np.concatenate([Mk.T, Mk.T], axis=1).astype(NPBF16))       # [H,128]
    ck_dup = np.tile(ck, 2).astype(np.float32).reshape(P, 1)
    W2T = np.ascontiguousarray(
        (fc2_w.T[None] * g1[:, :, None]).astype(NPBF16))           # [T,A,H]
    WvT = np.ascontiguousarray((v_w * gv[:, None]).T.astype(NPBF16))  # [H,H]
    vbg_perm = (v_b * gv).reshape(NH, HD).T.reshape(H)             # h' = d*16+n
    fc1T = np.ascontiguousarray(fc1_w.T.astype(NPBF16))            # [H,A]
    fc1b = fc1_b.astype(np.float32).reshape(NAC, P, 1)
    fc2b = fc2_b.astype(np.float32).reshape(NHC, P, 1)
    # g2sb[r, j*T+p] = g2[p, j*128+r]
    g2sb = np.ascontiguousarray(
        g2.reshape(T, NHC, P).transpose(2, 1, 0).reshape(P, NHC * T).astype(np.float32))

    shared = dict(fc1T=fc1T, fc1b=fc1b, W2T=W2T, fc2b=fc2b,
                  MkT=MkTdup, ck=ck_dup, WvT=WvT, g2sb=g2sb)
    per_core = []
    x32 = np.asarray(x, np.float32)
    xres_all = x32 + vbg_perm.astype(np.float32)[None, None, :]
    for b_ in range(B):
        m = dict(shared)
        m["xT"] = np.ascontiguousarray(x32[b_].T.astype(NPBF16))
        m["xres"] = np.ascontiguousarray(xres_all[b_])
        per_core.append(m)
    return per_core


def kernel(**inputs):
    if "nc" not in _CACHE:
        _CACHE["nc"] = _build_nc()
    nc = _CACHE["nc"]
    in_maps = _host_prep(**inputs)
    last_err = None
    for _attempt in range(3):
        try:
            res = run_bass_kernel_spmd(nc, in_maps, core_ids=list(range(B)))
            break
        except Exception as e:  # transient NRT device errors: retry
            last_err = e
    else:
        raise last_err
    out = np.stack([res.results[c]["out"] for c in range(B)], axis=0)
    return out.astype(np.float32)


# revision 14
# speedup vs baseline: 3.0115x; 1.0523x over previous
"""Trainium2 Bass kernel for nn_BertAdapterAttentionMask.

Math restructuring (validated vs reference in fp64):
  * The query comes from a broadcast task embedding -> q is one [H] vector,
    constant over (b, s). The K projection therefore collapses to a rank-hd
    GEMM:  scores[., d] = hid @ Mk[d, :] + ck[d], Mk = fold(q, gk, k_w)/sqrt(hd).
  * Per-task adapter gates fold into fc2 weights (host side).
  * softmax(scores) sums to 1 over tasks, so the V bias contributes a constant
    vector; it is folded into the residual x on the host.
  * scores GEMM uses a column-duplicated Mk (M=128) so probs come out
    partition-duplicated for free (broadcast over the 2x64 row halves).

Per-core layout: fully "transposed" pipeline (features on partitions,
sequence on the free axis). Data-parallel over batch B=8 across 8 cores.
All GEMM operands bf16 (1 cyc/row on PE), fp32 accumulation + softmax.
"""

import numpy as np
import ml_dtypes
from contextlib import ExitStack

import concourse.bass as bass
import concourse.tile as tile
from concourse import bacc, mybir
from concourse.bass_utils import run_bass_kernel_spmd
from concourse.masks import make_identity

AF = mybir.ActivationFunctionType
BF16 = mybir.dt.bfloat16
F32 = mybir.dt.float32
NPBF16 = ml_dtypes.bfloat16

B, S, H, A, NH, HD = 8, 2048, 1024, 512, 16, 64
T = 6              # tasks = t + 1
P = 128
ST = 512           # s-tile (free-dim tile)
NST = S // ST      # 4
NHC = H // P       # 8 h-chunks
NAC = A // P       # 4 a-chunks
SMAX = 400.0

_CACHE = {}


def _build_nc():
    nc = bacc.Bacc("TRN2", target_bir_lowering=False, debug=False)

    d_xT = nc.dram_tensor("xT", [H, S], BF16, kind="ExternalInput").ap()
    d_xres = nc.dram_tensor("xres", [S, H], F32, kind="ExternalInput").ap()
    d_fc1T = nc.dram_tensor("fc1T", [H, A], BF16, kind="ExternalInput").ap()
    d_fc1b = nc.dram_tensor("fc1b", [NAC, P, 1], F32, kind="ExternalInput").ap()
    d_W2T = nc.dram_tensor("W2T", [T, A, H], BF16, kind="ExternalInput").ap()
    d_fc2b = nc.dram_tensor("fc2b", [NHC, P, 1], F32, kind="ExternalInput").ap()
    d_Mk = nc.dram_tensor("MkT", [H, P], BF16, kind="ExternalInput").ap()
    d_ck = nc.dram_tensor("ck", [P, 1], F32, kind="ExternalInput").ap()
    d_Wv = nc.dram_tensor("WvT", [H, H], BF16, kind="ExternalInput").ap()
    d_g2 = nc.dram_tensor("g2sb", [P, NHC * T], F32, kind="ExternalInput").ap()
    d_out = nc.dram_tensor("out", [S, H], F32, kind="ExternalOutput").ap()

    with tile.TileContext(nc) as tc:
        with ExitStack() as ctx:
            wp = ctx.enter_context(tc.tile_pool(name="weights", bufs=1))
            xp = ctx.enter_context(tc.tile_pool(name="acts", bufs=2))
            psp = ctx.enter_context(
                tc.tile_pool(name="psum", bufs=2, space="PSUM")
            )

            # ---- resident weights (DMA order = first-use order: fc1 deps
            # first so the PE can start within a few us) ----
            w1 = []
            for k in range(NHC):
                t_ = wp.tile([P, A], BF16, tag=f"w1_{k}")
                nc.sync.dma_start(t_[:], d_fc1T[k * P:(k + 1) * P, :])
                w1.append(t_)
            b1 = wp.tile([P, NAC], F32, tag="b1")
            for ac in range(NAC):
                nc.sync.dma_start(b1[:, ac:ac + 1], d_fc1b[ac])
            xt0 = []
            for k in range(NHC):
                t_ = xp.tile([P, ST], BF16, name=f"xt{k}", tag=f"xt_{k}", bufs=1)
                nc.sync.dma_start(t_[:], d_xT[k * P:(k + 1) * P, 0:ST])
                xt0.append(t_)
            b2 = wp.tile([P, NHC], F32, tag="b2")
            for hc in range(NHC):
                nc.sync.dma_start(b2[:, hc:hc + 1], d_fc2b[hc])
            g2t = wp.tile([P, NHC * T], F32, tag="g2")
            nc.sync.dma_start(g2t[:], d_g2[:])
            w2 = [[None] * NAC for _ in range(T)]
            for p in range(T):
                for ac in range(NAC):
                    t_ = wp.tile([P, H], BF16, tag=f"w2_{p}_{ac}")
                    nc.sync.dma_start(t_[:], d_W2T[p, ac * P:(ac + 1) * P, :])
                    w2[p][ac] = t_
            wmk = []
            for j in range(NHC):
                t_ = wp.tile([P, P], BF16, tag=f"wmk_{j}")
                nc.sync.dma_start(t_[:], d_Mk[j * P:(j + 1) * P, :])
                wmk.append(t_)
            ckt = wp.tile([P, 1], F32, tag="ck")
            nc.sync.dma_start(ckt[:], d_ck[:])
            wv = []
            for j in range(NHC):
                t_ = wp.tile([P, H], BF16, tag=f"wv_{j}")
                nc.sync.dma_start(t_[:], d_Wv[j * P:(j + 1) * P, :])
                wv.append(t_)
            ident = wp.tile([P, P], BF16, tag="ident")
            make_identity(nc, ident[:])

            pending_E = []   # deferred phase-E emitters (overlap next fc1)
            for st in range(NST):
                s0 = st * ST
                # ---- load xT chunks ----
                if st == 0:
                    xt = xt0
                else:
                    xt = []
                    for k in range(NHC):
                        t_ = xp.tile([P, ST], BF16, name=f"xt{k}", tag=f"xt_{k}", bufs=1)
                        nc.sync.dma_start(t_[:], d_xT[k * P:(k + 1) * P, s0:s0 + ST])
                        xt.append(t_)
                # ---- fc1 -> h1T (gelu) ----
                h1 = []
                for ac in range(NAC):
                    ps = psp.tile([P, ST], F32, tag="ps_mm", bufs=3)
                    for k in range(NHC):
                        nc.tensor.matmul(
                            ps[:], w1[k][:, ac * P:(ac + 1) * P], xt[k][:],
                            start=(k == 0), stop=(k == NHC - 1),
                        )
                    t_ = xp.tile([P, ST], BF16, name=f"h1_{ac}", tag=f"h1_{ac}", bufs=2)
                    nc.scalar.activation(t_[:], ps[:], AF.Gelu, bias=b1[:, ac:ac + 1])
                    h1.append(t_)
                # flush previous s-tile's deferred phase-E (overlaps fc2 GEMMs)
                for fn in pending_E:
                    fn()
                pending_E = []
                # ---- fc2 per task -> gated gelu store ----
                gst = [
                    xp.tile([P, T, ST], BF16, name=f"g{j}", tag=f"g_{j}", bufs=1)
                    for j in range(NHC)
                ]
                for p in range(T):
                    for j in range(NHC):
                        ps = psp.tile([P, ST], F32, tag="ps_mm", bufs=3)
                        for ac in range(NAC):
                            nc.tensor.matmul(
                                ps[:], w2[p][ac][:, j * P:(j + 1) * P], h1[ac][:],
                                start=(ac == 0), stop=(ac == NAC - 1),
                            )
                        nc.scalar.activation(
                            gst[j][:, p, :], ps[:], AF.Gelu, bias=b2[:, j:j + 1]
                        )
                        nc.vector.tensor_scalar_mul(
                            gst[j][:, p, :], gst[j][:, p, :],
                            g2t[:, j * T + p:j * T + p + 1],
                        )
                # ---- scores (batched: one ACT table switch) -> e = exp ----
                e_t = xp.tile([P, T, ST], F32, tag="e", bufs=1)
                for p in range(T):
                    ps_s = psp.tile([P, ST], F32, tag="ps_s", bufs=1)
                    for j in range(NHC):
                        nc.tensor.matmul(
                            ps_s[:], wmk[j][:], gst[j][:, p, :],
                            start=(j == 0), stop=(j == NHC - 1),
                        )
                    nc.scalar.activation(e_t[:, p, :], ps_s[:], AF.Exp, bias=ckt[:])
                # ---- softmax over tasks (no max-sub: |scores| < 1) ----
                d0 = xp.tile([P, ST], F32, tag="den", bufs=3)
                d1 = xp.tile([P, ST], F32, tag="den", bufs=3)
                d2 = xp.tile([P, ST], F32, tag="den", bufs=3)
                nc.vector.tensor_add(d0[:], e_t[:, 0, :], e_t[:, 1, :])
                nc.vector.tensor_add(d1[:], e_t[:, 2, :], e_t[:, 3, :])
                nc.vector.tensor_add(d2[:], e_t[:, 4, :], e_t[:, 5, :])
                nc.vector.tensor_add(d0[:], d0[:], d1[:])
                nc.vector.tensor_add(d0[:], d0[:], d2[:])
                nc.vector.reciprocal(d0[:], d0[:])
                # ---- V GEMM + probs-weighted task sum + transpose/store ----
                # phase E (transpose + head-permute + residual) for each
                # 4-chunk half is emitted late so the PE overlaps it with
                # later GEMM work: half1-E after half2's V MMs, half2-E after
                # the NEXT s-tile's fc1 (via pending_E).
                xrs, ots = [], []
                for sb in range(ST // P):
                    r0 = s0 + sb * P
                    xr = xp.tile([P, H], F32, name=f"xr{sb}", tag=f"xr_{sb}", bufs=1)
                    nc.sync.dma_start(xr[:], d_xres[r0:r0 + P, :])
                    ot = xp.tile([P, H], F32, name=f"ot{sb}", tag=f"ot_{sb}", bufs=1)
                    xrs.append(xr)
                    ots.append(ot)

                def emit_E(ctxs, h2, ots=ots, xrs=xrs, s0=s0, last=False):
                    for sb in range(ST // P):
                        ps_t = psp.tile([P, ST], BF16, tag="ps_t", name="ps_t")
                        for q in range(4):
                            nc.tensor.transpose(
                                ps_t[:, q * P:(q + 1) * P],
                                ctxs[q][:, sb * P:(sb + 1) * P],
                                ident[:],
                            )
                        # out cols h' = d*16 + h2*8 + c*2 + nl for psum (c,nl,d)
                        o_ap = ots[sb][:].rearrange(
                            "p (d h2 c nl) -> p h2 c nl d", d=HD, h2=2, c=4, nl=2
                        )[:, h2]
                        x_ap = xrs[sb][:].rearrange(
                            "p (d h2 c nl) -> p h2 c nl d", d=HD, h2=2, c=4, nl=2
                        )[:, h2]
                        p_ap = ps_t[:].rearrange("p (c nl d) -> p c nl d", c=4, nl=2, d=HD)
                        nc.vector.tensor_add(o_ap, p_ap, x_ap)
                        if last:
                            nc.sync.dma_start(
                                d_out[s0 + sb * P:s0 + (sb + 1) * P, :], ots[sb][:]
                            )

                halves = []
                for h2 in range(2):
                    ctxs = []
                    for q in range(4):
                        hc = h2 * 4 + q
                        eng = nc.gpsimd if q < 2 else nc.vector
                        sc = []
                        for p in range(T):
                            ps_v = psp.tile([P, ST], F32, tag="ps_v")
                            for j in range(NHC):
                                nc.tensor.matmul(
                                    ps_v[:], wv[j][:, hc * P:(hc + 1) * P],
                                    gst[j][:, p, :],
                                    start=(j == 0), stop=(j == NHC - 1),
                                )
                            t_ = xp.tile([P, ST], BF16, name=f"sc{p}", tag="sc", bufs=8)
                            nc.vector.tensor_mul(t_[:], ps_v[:], e_t[:, p, :])
                            sc.append(t_)
                        eng.tensor_add(sc[0][:], sc[0][:], sc[1][:])
                        eng.tensor_add(sc[2][:], sc[2][:], sc[3][:])
                        eng.tensor_add(sc[4][:], sc[4][:], sc[5][:])
                        eng.tensor_add(sc[0][:], sc[0][:], sc[2][:])
                        eng.tensor_add(sc[0][:], sc[0][:], sc[4][:])
                        cx = xp.tile([P, ST], BF16, tag="ctx", bufs=10)
                        eng.tensor_mul(cx[:], sc[0][:], d0[:])
                        ctxs.append(cx)
                    halves.append(ctxs)
                emit_E(halves[0], 0)
                pending_E.append(lambda e=emit_E, c=halves[1]: e(c, 1, last=True))
            for fn in pending_E:
                fn()
            pending_E = []
    nc.compile()
    return nc


def _sigmoid(x):
    with np.errstate(over="ignore"):
        return 1.0 / (1.0 + np.exp(-x))


def _host_prep(x, fc1_w, fc1_b, fc2_w, fc2_b, efc1, efc2, etask,
               q_w, q_b, k_w, k_b, v_w, v_b, equery, ekey, evalue, t, s):
    f64 = np.float64
    t = int(t)
    s = float(s)
    assert t + 1 == T and x.shape == (B, S, H)
    fc1_w = np.asarray(fc1_w, f64); fc1_b = np.asarray(fc1_b, f64)
    fc2_w = np.asarray(fc2_w, f64); fc2_b = np.asarray(fc2_b, f64)
    efc1 = np.asarray(efc1, f64); efc2 = np.asarray(efc2, f64)
    etask = np.asarray(etask, f64)
    q_w = np.asarray(q_w, f64); q_b = np.asarray(q_b, f64)
    k_w = np.asarray(k_w, f64); k_b = np.asarray(k_b, f64)
    v_w = np.asarray(v_w, f64); v_b = np.asarray(v_b, f64)
    equery = np.asarray(equery, f64); ekey = np.asarray(ekey, f64)
    evalue = np.asarray(evalue, f64)

    g1 = np.stack([_sigmoid(s * efc1[t])] + [_sigmoid(SMAX * efc1[p]) for p in range(t)])
    g2 = np.stack([_sigmoid(s * efc2[t])] + [_sigmoid(SMAX * efc2[p]) for p in range(t)])
    gq = _sigmoid(s * equery[t]); gk = _sigmoid(s * ekey[t]); gv = _sigmoid(s * evalue[t])

    q_vec = (etask[t] @ q_w.T + q_b) * gq
    q_mat = q_vec.reshape(NH, HD)
    kwg = k_w * gk[:, None]
    Mk = np.einsum("nd,ndj->dj", q_mat, kwg.reshape(NH, HD, H)) / np.sqrt(HD)
    ck = np.einsum("nd,nd->d", q_mat, (k_b * gk).reshape(NH, HD)) / np.sqrt(HD)

    MkTdup = np.ascontiguousarray(
        np.concatenate([Mk.T, Mk.T], axis=1).astype(NPBF16))       # [H,128]
    ck_dup = np.tile(ck, 2).astype(np.float32).reshape(P, 1)
    W2T = np.ascontiguousarray(
        (fc2_w.T[None] * g1[:, :, None]).astype(NPBF16))           # [T,A,H]
    WvT = np.ascontiguousarray((v_w * gv[:, None]).T.astype(NPBF16))  # [H,H]
    vbg_perm = (v_b * gv).reshape(NH, HD).T.reshape(H)             # h' = d*16+n
    fc1T = np.ascontiguousarray(fc1_w.T.astype(NPBF16))            # [H,A]
    fc1b = fc1_b.astype(np.float32).reshape(NAC, P, 1)
    fc2b = fc2_b.astype(np.float32).reshape(NHC, P, 1)
    # g2sb[r, j*T+p] = g2[p, j*128+r]
    g2sb = np.ascontiguousarray(
        g2.reshape(T, NHC, P).transpose(2, 1, 0).reshape(P, NHC * T).astype(np.float32))

    shared = dict(fc1T=fc1T, fc1b=fc1b, W2T=W2T, fc2b=fc2b,
                  MkT=MkTdup, ck=ck_dup, WvT=WvT, g2sb=g2sb)
    per_core = []
    x32 = np.asarray(x, np.float32)
    xres_all = x32 + vbg_perm.astype(np.float32)[None, None, :]
    for b_ in range(B):
        m = dict(shared)
        m["xT"] = np.ascontiguousarray(x32[b_].T.astype(NPBF16))
        m["xres"] = np.ascontiguousarray(xres_all[b_])
        per_core.append(m)
    return per_core


def kernel(**inputs):
    if "nc" not in _CACHE:
        _CACHE["nc"] = _build_nc()
    nc = _CACHE["nc"]
    in_maps = _host_prep(**inputs)
    last_err = None
    for _attempt in range(3):
        try:
            res = run_bass_kernel_spmd(nc, in_maps, core_ids=list(range(B)))
            break
        except Exception as e:  # transient NRT device errors: retry
            last_err = e
    else:
        raise last_err
    out = np.stack([res.results[c]["out"] for c in range(B)], axis=0)
    return out.astype(np.float32)


# revision 15
# speedup vs baseline: 3.0644x; 1.0176x over previous
"""Trainium2 Bass kernel for nn_BertAdapterAttentionMask.

Math restructuring (validated vs reference in fp64):
  * The query comes from a broadcast task embedding -> q is one [H] vector,
    constant over (b, s). The K projection therefore collapses to a rank-hd
    GEMM:  scores[., d] = hid @ Mk[d, :] + ck[d], Mk = fold(q, gk, k_w)/sqrt(hd).
  * Per-task adapter gates fold into fc2 weights (host side).
  * softmax(scores) sums to 1 over tasks, so the V bias contributes a constant
    vector; it is folded into the residual x on the host.
  * scores GEMM uses a column-duplicated Mk (M=128) so probs come out
    partition-duplicated for free (broadcast over the 2x64 row halves).

Per-core layout: fully "transposed" pipeline (features on partitions,
sequence on the free axis). Data-parallel over batch B=8 across 8 cores.
All GEMM operands bf16 (1 cyc/row on PE), fp32 accumulation + softmax.
"""

import numpy as np
import ml_dtypes
from contextlib import ExitStack

import concourse.bass as bass
import concourse.tile as tile
from concourse import bacc, mybir
from concourse.bass_utils import run_bass_kernel_spmd
from concourse.masks import make_identity

AF = mybir.ActivationFunctionType
BF16 = mybir.dt.bfloat16
F32 = mybir.dt.float32
NPBF16 = ml_dtypes.bfloat16

B, S, H, A, NH, HD = 8, 2048, 1024, 512, 16, 64
T = 6              # tasks = t + 1
P = 128
ST = 512           # s-tile (free-dim tile)
NST = S // ST      # 4
NHC = H // P       # 8 h-chunks
NAC = A // P       # 4 a-chunks
SMAX = 400.0

_CACHE = {}


def _build_nc():
    nc = bacc.Bacc("TRN2", target_bir_lowering=False, debug=False)

    d_xT = nc.dram_tensor("xT", [H, S], BF16, kind="ExternalInput").ap()
    d_xres = nc.dram_tensor("xres", [S, H], F32, kind="ExternalInput").ap()
    d_fc1T = nc.dram_tensor("fc1T", [H, A], BF16, kind="ExternalInput").ap()
    d_fc1b = nc.dram_tensor("fc1b", [NAC, P, 1], F32, kind="ExternalInput").ap()
    d_W2T = nc.dram_tensor("W2T", [T, A, H], BF16, kind="ExternalInput").ap()
    d_fc2b = nc.dram_tensor("fc2b", [NHC, P, 1], F32, kind="ExternalInput").ap()
    d_Mk = nc.dram_tensor("MkT", [H, P], BF16, kind="ExternalInput").ap()
    d_ck = nc.dram_tensor("ck", [P, 1], F32, kind="ExternalInput").ap()
    d_Wv = nc.dram_tensor("WvT", [H, H], BF16, kind="ExternalInput").ap()
    d_g2 = nc.dram_tensor("g2sb", [P, NHC * T], F32, kind="ExternalInput").ap()
    d_out = nc.dram_tensor("out", [S, H], F32, kind="ExternalOutput").ap()

    with tile.TileContext(nc) as tc:
        with ExitStack() as ctx:
            wp = ctx.enter_context(tc.tile_pool(name="weights", bufs=1))
            xp = ctx.enter_context(tc.tile_pool(name="acts", bufs=2))
            psp = ctx.enter_context(
                tc.tile_pool(name="psum", bufs=2, space="PSUM")
            )

            # ---- resident weights (DMA order = first-use order: fc1 deps
            # first so the PE can start within a few us) ----
            w1 = []
            xt0 = []
            for k in range(NHC):
                t_ = wp.tile([P, A], BF16, name=f"w1{k}", tag=f"w1_{k}")
                nc.sync.dma_start(t_[:], d_fc1T[k * P:(k + 1) * P, :])
                w1.append(t_)
                t_ = xp.tile([P, ST], BF16, name=f"xt{k}", tag=f"xt_{k}", bufs=1)
                nc.sync.dma_start(t_[:], d_xT[k * P:(k + 1) * P, 0:ST])
                xt0.append(t_)
            b1 = wp.tile([P, NAC], F32, tag="b1")
            for ac in range(NAC):
                nc.sync.dma_start(b1[:, ac:ac + 1], d_fc1b[ac])
            b2 = wp.tile([P, NHC], F32, tag="b2")
            for hc in range(NHC):
                nc.sync.dma_start(b2[:, hc:hc + 1], d_fc2b[hc])
            g2t = wp.tile([P, NHC * T], F32, tag="g2")
            nc.sync.dma_start(g2t[:], d_g2[:])
            w2 = [[None] * NAC for _ in range(T)]
            for p in range(T):
                for ac in range(NAC):
                    t_ = wp.tile([P, H], BF16, tag=f"w2_{p}_{ac}")
                    nc.sync.dma_start(t_[:], d_W2T[p, ac * P:(ac + 1) * P, :])
                    w2[p][ac] = t_
            wmk = []
            for j in range(NHC):
                t_ = wp.tile([P, P], BF16, tag=f"wmk_{j}")
                nc.sync.dma_start(t_[:], d_Mk[j * P:(j + 1) * P, :])
                wmk.append(t_)
            ckt = wp.tile([P, 1], F32, tag="ck")
            nc.sync.dma_start(ckt[:], d_ck[:])
            wv = []
            for j in range(NHC):
                t_ = wp.tile([P, H], BF16, tag=f"wv_{j}")
                nc.sync.dma_start(t_[:], d_Wv[j * P:(j + 1) * P, :])
                wv.append(t_)
            ident = wp.tile([P, P], BF16, tag="ident")
            make_identity(nc, ident[:])

            pending_E = []   # deferred phase-E emitters (overlap next fc1)
            for st in range(NST):
                s0 = st * ST
                # ---- load xT chunks ----
                if st == 0:
                    xt = xt0
                else:
                    xt = []
                    for k in range(NHC):
                        t_ = xp.tile([P, ST], BF16, name=f"xt{k}", tag=f"xt_{k}", bufs=1)
                        nc.sync.dma_start(t_[:], d_xT[k * P:(k + 1) * P, s0:s0 + ST])
                        xt.append(t_)
                # ---- fc1 -> h1T (gelu) ----
                h1 = []
                for ac in range(NAC):
                    ps = psp.tile([P, ST], F32, tag="ps_mm", bufs=3)
                    for k in range(NHC):
                        nc.tensor.matmul(
                            ps[:], w1[k][:, ac * P:(ac + 1) * P], xt[k][:],
                            start=(k == 0), stop=(k == NHC - 1),
                        )
                    t_ = xp.tile([P, ST], BF16, name=f"h1_{ac}", tag=f"h1_{ac}", bufs=2)
                    nc.scalar.activation(t_[:], ps[:], AF.Gelu, bias=b1[:, ac:ac + 1])
                    h1.append(t_)
                # flush previous s-tile's deferred phase-E (overlaps fc2 GEMMs)
                for fn in pending_E:
                    fn()
                pending_E = []
                # ---- fc2 per task -> gated gelu store ----
                gst = [
                    xp.tile([P, T, ST], BF16, name=f"g{j}", tag=f"g_{j}", bufs=1)
                    for j in range(NHC)
                ]
                for p in range(T):
                    for j in range(NHC):
                        ps = psp.tile([P, ST], F32, tag="ps_mm", bufs=3)
                        for ac in range(NAC):
                            nc.tensor.matmul(
                                ps[:], w2[p][ac][:, j * P:(j + 1) * P], h1[ac][:],
                                start=(ac == 0), stop=(ac == NAC - 1),
                            )
                        nc.scalar.activation(
                            gst[j][:, p, :], ps[:], AF.Gelu, bias=b2[:, j:j + 1]
                        )
                        nc.vector.tensor_scalar_mul(
                            gst[j][:, p, :], gst[j][:, p, :],
                            g2t[:, j * T + p:j * T + p + 1],
                        )
                # ---- scores (batched: one ACT table switch) -> e = exp ----
                e_t = xp.tile([P, T, ST], F32, tag="e", bufs=1)
                for p in range(T):
                    ps_s = psp.tile([P, ST], F32, tag="ps_st", bufs=2, name="ps_s")
                    for j in range(NHC):
                        nc.tensor.matmul(
                            ps_s[:], wmk[j][:], gst[j][:, p, :],
                            start=(j == 0), stop=(j == NHC - 1),
                        )
                    nc.scalar.activation(e_t[:, p, :], ps_s[:], AF.Exp, bias=ckt[:])
                # softmax denominator tiles (emitted later, inside V phase,
                # so the first evict-muls are not queued behind them on DVE)
                d0 = xp.tile([P, ST], F32, tag="den", bufs=3)
                d1 = xp.tile([P, ST], F32, tag="den", bufs=3)
                d2 = xp.tile([P, ST], F32, tag="den", bufs=3)

                def emit_den():
                    nc.vector.tensor_add(d0[:], e_t[:, 0, :], e_t[:, 1, :])
                    nc.vector.tensor_add(d1[:], e_t[:, 2, :], e_t[:, 3, :])
                    nc.vector.tensor_add(d2[:], e_t[:, 4, :], e_t[:, 5, :])
                    nc.vector.tensor_add(d0[:], d0[:], d1[:])
                    nc.vector.tensor_add(d0[:], d0[:], d2[:])
                    nc.vector.reciprocal(d0[:], d0[:])
                # ---- V GEMM + probs-weighted task sum + transpose/store ----
                # phase E (transpose + head-permute + residual) for each
                # 4-chunk half is emitted late so the PE overlaps it with
                # later GEMM work: half1-E after half2's V MMs, half2-E after
                # the NEXT s-tile's fc1 (via pending_E).
                xrs, ots = [], []
                for sb in range(ST // P):
                    r0 = s0 + sb * P
                    xr = xp.tile([P, H], F32, name=f"xr{sb}", tag=f"xr_{sb}", bufs=1)
                    nc.sync.dma_start(xr[:], d_xres[r0:r0 + P, :])
                    ot = xp.tile([P, H], F32, name=f"ot{sb}", tag=f"ot_{sb}", bufs=1)
                    xrs.append(xr)
                    ots.append(ot)

                def emit_E(ctxs, h2, ots=ots, xrs=xrs, s0=s0, last=False):
                    for sb in range(ST // P):
                        ps_t = psp.tile([P, ST], BF16, tag="ps_st", bufs=2, name="ps_t")
                        for q in range(4):
                            nc.tensor.transpose(
                                ps_t[:, q * P:(q + 1) * P],
                                ctxs[q][:, sb * P:(sb + 1) * P],
                                ident[:],
                            )
                        # out cols h' = d*16 + h2*8 + c*2 + nl for psum (c,nl,d)
                        o_ap = ots[sb][:].rearrange(
                            "p (d h2 c nl) -> p h2 c nl d", d=HD, h2=2, c=4, nl=2
                        )[:, h2]
                        x_ap = xrs[sb][:].rearrange(
                            "p (d h2 c nl) -> p h2 c nl d", d=HD, h2=2, c=4, nl=2
                        )[:, h2]
                        p_ap = ps_t[:].rearrange("p (c nl d) -> p c nl d", c=4, nl=2, d=HD)
                        nc.vector.tensor_add(o_ap, p_ap, x_ap)
                        if last:
                            nc.sync.dma_start(
                                d_out[s0 + sb * P:s0 + (sb + 1) * P, :], ots[sb][:]
                            )

                halves = []
                for h2 in range(2):
                    ctxs = []
                    for q in range(4):
                        hc = h2 * 4 + q
                        eng = nc.gpsimd if q < 2 else nc.vector
                        sc = []
                        for p in range(T):
                            ps_v = psp.tile([P, ST], F32, tag="ps_v", bufs=3)
                            for j in range(NHC):
                                nc.tensor.matmul(
                                    ps_v[:], wv[j][:, hc * P:(hc + 1) * P],
                                    gst[j][:, p, :],
                                    start=(j == 0), stop=(j == NHC - 1),
                                )
                            t_ = xp.tile([P, ST], BF16, name=f"sc{p}", tag="sc", bufs=8)
                            nc.vector.tensor_mul(t_[:], ps_v[:], e_t[:, p, :])
                            sc.append(t_)
                        if h2 == 0 and q == 0:
                            emit_den()
                        eng.tensor_add(sc[0][:], sc[0][:], sc[1][:])
                        eng.tensor_add(sc[2][:], sc[2][:], sc[3][:])
                        eng.tensor_add(sc[4][:], sc[4][:], sc[5][:])
                        eng.tensor_add(sc[0][:], sc[0][:], sc[2][:])
                        eng.tensor_add(sc[0][:], sc[0][:], sc[4][:])
                        cx = xp.tile([P, ST], BF16, tag="ctx", bufs=10)
                        eng.tensor_mul(cx[:], sc[0][:], d0[:])
                        ctxs.append(cx)
                    halves.append(ctxs)
                emit_E(halves[0], 0)
                pending_E.append(lambda e=emit_E, c=halves[1]: e(c, 1, last=True))
            for fn in pending_E:
                fn()
            pending_E = []
    nc.compile()
    return nc


def _sigmoid(x):
    with np.errstate(over="ignore"):
        return 1.0 / (1.0 + np.exp(-x))


def _host_prep(x, fc1_w, fc1_b, fc2_w, fc2_b, efc1, efc2, etask,
               q_w, q_b, k_w, k_b, v_w, v_b, equery, ekey, evalue, t, s):
    f64 = np.float64
    t = int(t)
    s = float(s)
    assert t + 1 == T and x.shape == (B, S, H)
    fc1_w = np.asarray(fc1_w, f64); fc1_b = np.asarray(fc1_b, f64)
    fc2_w = np.asarray(fc2_w, f64); fc2_b = np.asarray(fc2_b, f64)
    efc1 = np.asarray(efc1, f64); efc2 = np.asarray(efc2, f64)
    etask = np.asarray(etask, f64)
    q_w = np.asarray(q_w, f64); q_b = np.asarray(q_b, f64)
    k_w = np.asarray(k_w, f64); k_b = np.asarray(k_b, f64)
    v_w = np.asarray(v_w, f64); v_b = np.asarray(v_b, f64)
    equery = np.asarray(equery, f64); ekey = np.asarray(ekey, f64)
    evalue = np.asarray(evalue, f64)

    g1 = np.stack([_sigmoid(s * efc1[t])] + [_sigmoid(SMAX * efc1[p]) for p in range(t)])
    g2 = np.stack([_sigmoid(s * efc2[t])] + [_sigmoid(SMAX * efc2[p]) for p in range(t)])
    gq = _sigmoid(s * equery[t]); gk = _sigmoid(s * ekey[t]); gv = _sigmoid(s * evalue[t])

    q_vec = (etask[t] @ q_w.T + q_b) * gq
    q_mat = q_vec.reshape(NH, HD)
    kwg = k_w * gk[:, None]
    Mk = np.einsum("nd,ndj->dj", q_mat, kwg.reshape(NH, HD, H)) / np.sqrt(HD)
    ck = np.einsum("nd,nd->d", q_mat, (k_b * gk).reshape(NH, HD)) / np.sqrt(HD)

    MkTdup = np.ascontiguousarray(
        np.concatenate([Mk.T, Mk.T], axis=1).astype(NPBF16))       # [H,128]
    ck_dup = np.tile(ck, 2).astype(np.float32).reshape(P, 1)
    W2T = np.ascontiguousarray(
        (fc2_w.T[None] * g1[:, :, None]).astype(NPBF16))           # [T,A,H]
    WvT = np.ascontiguousarray((v_w * gv[:, None]).T.astype(NPBF16))  # [H,H]
    vbg_perm = (v_b * gv).reshape(NH, HD).T.reshape(H)             # h' = d*16+n
    fc1T = np.ascontiguousarray(fc1_w.T.astype(NPBF16))            # [H,A]
    fc1b = fc1_b.astype(np.float32).reshape(NAC, P, 1)
    fc2b = fc2_b.astype(np.float32).reshape(NHC, P, 1)
    # g2sb[r, j*T+p] = g2[p, j*128+r]
    g2sb = np.ascontiguousarray(
        g2.reshape(T, NHC, P).transpose(2, 1, 0).reshape(P, NHC * T).astype(np.float32))

    shared = dict(fc1T=fc1T, fc1b=fc1b, W2T=W2T, fc2b=fc2b,
                  MkT=MkTdup, ck=ck_dup, WvT=WvT, g2sb=g2sb)
    per_core = []
    x32 = np.asarray(x, np.float32)
    xres_all = x32 + vbg_perm.astype(np.float32)[None, None, :]
    for b_ in range(B):
        m = dict(shared)
        m["xT"] = np.ascontiguousarray(x32[b_].T.astype(NPBF16))
        m["xres"] = np.ascontiguousarray(xres_all[b_])
        per_core.append(m)
    return per_core


def kernel(**inputs):
    if "nc" not in _CACHE:
        _CACHE["nc"] = _build_nc()
    nc = _CACHE["nc"]
    in_maps = _host_prep(**inputs)
    last_err = None
    for _attempt in range(3):
        try:
            res = run_bass_kernel_spmd(nc, in_maps, core_ids=list(range(B)))
            break
        except Exception as e:  # transient NRT device errors: retry
            last_err = e
    else:
        raise last_err
    out = np.stack([res.results[c]["out"] for c in range(B)], axis=0)
    return out.astype(np.float32)
